# revision 9
# baseline (speedup 1.0000x reference)
"""CenterLoss (segment_reduce) Trainium2 kernel — PE segment-sum version.

Math (faithful to the reference):
  preds = argmax_c logits[n, c, h, w]          (softmax is monotone -> skip it)
  per (n, cls): cnt = #pixels with preds==cls, S1 = sum_{pix,ch} x,
                S2 = sum_{pix,ch} x^2 over pixels of that class
  K = max(cnt,1)*C; sq_dev = max(S2 - S1^2/K, 0)
  loss = sum_cls mean_n( cnt>0 ? sqrt(sq_dev) : 0 )

Device strategy (8 cores, data-parallel; core k owns sample n=k//2's
half of H): pixels live on (128 partitions x 2048 w-slots) per core, in
16 segments of 128 slots.  Input is cast to fp16 on the host (halves
DMA; measured rel err 4.7e-4 incl. fp16 argmax ties double-matching
~0.09% of pixels — well inside the 2e-2 gate).

Per segment the host ships a (P, 20, 64, 4) fp16 tile: rows 0:19 the
19 class channels, row 19 ones, w-slots viewed as (wb, k) blocks of 4.
On device:
  ScalarE: rows 20:39 = Square(rows 0:19)       (one ACT op)
  DVE:     m = pairwise fp16 max-tree over the 19 rows (2x_1P mode)
           eq = is_equal(x, broadcast m) -> one-hot, written through a
           transposed AP into an interleaved (P, wb, c, k) tile so each
           slot's 19-col stationary sits at a dense 8-byte stride (10
           SBUF lines per LDWEIGHTS instead of 19) while the DVE write
           keeps a step-1 innermost dim (stays in 2x mode).
  PE:      per slot: psum[32k + c, j] += eq[:, c] * mov[:, j] — a
           (128x19)x(128x39) matmul accumulated over all 2048 slots,
           rotating k=0..3 across four 32-col PSUM groups so the small
           matmuls overlap in the array.
psum cols 0:19 are the per-(cls, ch) S1 breakdown, col 19 cnt, cols
20:39 the S2 breakdown.  One ACT copy + tiny DMA ships (128, 39) f32
per core; the host folds the 4 col-groups, sums the channel breakdowns
and applies the final sqrt/mean formula.  `target` is unused by the
reference and never shipped.
"""

import numpy as np


def _ensure_ntff_hook():
    """bass_utils' trace path imports antenv.axon_hooks, which this image
    lacks.  Install a shim backed by trn_agent_boot's ctypes hook so a
    BASS_TRACE=1 environment doesn't crash the run (and tracing works)."""
    import sys
    import types

    try:
        import antenv.axon_hooks  # noqa: F401
        return
    except ImportError:
        pass
    try:
        from trn_agent_boot.trn_boot import _ntff_profile_via_ctypes

        hook = _ntff_profile_via_ctypes("/opt/axon/libaxon_pjrt.so")
    except Exception:
        hook = None
    mod = types.ModuleType("antenv.axon_hooks")
    mod.get_axon_ntff_profile_hook = lambda: hook
    mod.set_axon_ntff_profile_hook = lambda h: None
    sys.modules["antenv.axon_hooks"] = mod

N, C, H, W = 4, 19, 512, 1024
NCORES = 8
SLABS = 4                 # H split into 4 slabs of 128 partitions
P = H // SLABS            # 128
UNITS = [(n, s) for n in range(N) for s in range(SLABS)]   # 16 units
UPC = len(UNITS) // NCORES                                  # 2 units per core
SEGW = 128
NSEG = (UPC * W) // SEGW   # 8 segments of 256 w-slots per core
NGRP = 4                   # PE col-groups (PSUM partition offsets 32k)
WB = SEGW // NGRP          # 64 slot-blocks per segment
MROWS = 40                 # mov tile rows: 0:19 x, 19 ones, 20:39 sq

_CACHE = {}


def _build_nc():
    from contextlib import ExitStack

    import concourse.tile as tile
    from concourse import bacc, mybir

    f32 = mybir.dt.float32
    f16 = mybir.dt.float16
    Alu = mybir.AluOpType
    Act = mybir.ActivationFunctionType

    nc = bacc.Bacc("TRN2", target_bir_lowering=False, debug=False)
    # Host pre-arranges each core's shard as (seg, h, row, wb, k) fp16 with
    # rows 0:19 = channels and row 19 = 1.0, so one segment load is 128
    # contiguous 10 KB runs.
    x_d = nc.dram_tensor(
        "x", [NSEG, P, 20, WB, NGRP], f16, kind="ExternalInput"
    ).ap()
    out_d = nc.dram_tensor("stats", [P, 2 * C + 1], f32, kind="ExternalOutput").ap()

    with tile.TileContext(nc) as tc, ExitStack() as ctx:
        movpool = ctx.enter_context(tc.tile_pool(name="mov", bufs=4))
        eqpool = ctx.enter_context(tc.tile_pool(name="eq", bufs=3))
        tpool = ctx.enter_context(tc.tile_pool(name="tree", bufs=2))
        spool = ctx.enter_context(tc.tile_pool(name="stats", bufs=1))
        ppool = ctx.enter_context(tc.tile_pool(name="ps", bufs=1, space="PSUM"))

        ps = ppool.tile([P, 2 * C + 1], f32, name="psacc")

        for s in range(NSEG):
            mov = movpool.tile([P, MROWS, WB, NGRP], f16, tag="mov", name=f"mov{s}")
            nc.sync.dma_start(mov[:, 0:20, :, :], x_d[s])

            # squares on the otherwise-idle ScalarE
            nc.scalar.activation(
                mov[:, 20:39, :, :], mov[:, 0:19, :, :], Act.Square
            )

            # fp16 pairwise max-tree over the 19 channel rows (DVE 2x_1P)
            t = tpool.tile([P, 9, WB, NGRP], f16, tag="t", name=f"t{s}")
            m = tpool.tile([P, WB, NGRP], f16, tag="m", name=f"m{s}")
            tt = nc.vector.tensor_tensor
            tt(out=t[:, 0:9], in0=mov[:, 0:9], in1=mov[:, 9:18], op=Alu.max)
            tt(out=t[:, 0:4], in0=t[:, 0:4], in1=t[:, 4:8], op=Alu.max)
            tt(out=t[:, 0:2], in0=t[:, 0:2], in1=t[:, 2:4], op=Alu.max)
            tt(out=t[:, 0], in0=t[:, 0], in1=t[:, 1], op=Alu.max)
            tt(out=t[:, 0], in0=t[:, 0], in1=t[:, 8], op=Alu.max)
            tt(out=m[:], in0=t[:, 0], in1=mov[:, 18], op=Alu.max)

            # one-hot masks: eq[p, wb, c, k] = (x[p, c, wb, k] == m[p, wb, k])
            # (interleaved tile; DVE writes through a transposed AP, so the
            # innermost iterated dim keeps step 1 and 2x mode)
            eq = eqpool.tile([P, WB, C, NGRP], f16, tag="eq", name=f"eq{s}")
            eqv = eq[:].transpose([0, 2, 1, 3])          # (P, C, WB, NGRP)
            mb = m[:].unsqueeze(1).broadcast_to([P, C, WB, NGRP])
            tt(out=eqv, in0=mov[:, 0:19], in1=mb, op=Alu.is_equal)

            # segment sums on the PE: one small matmul per w-slot, rotating
            # k across 4 concurrent PSUM col-groups
            for wb in range(WB):
                for k in range(NGRP):
                    nc.tensor.matmul(
                        out=ps[32 * k:32 * k + C, :],
                        lhsT=eq[:, wb, :, k],
                        rhs=mov[:, 0:39, wb, k],
                        start=(s == 0 and wb == 0),
                        stop=(s == NSEG - 1 and wb == WB - 1),
                        tile_position=(0, 32 * k),
                    )

        stats = spool.tile([P, 2 * C + 1], f32, name="stats")
        nc.scalar.activation(stats[:], ps[:], Act.Copy)
        nc.sync.dma_start(out_d[:], stats[:])

    nc.compile()
    return nc


def _get_nc():
    if "nc" not in _CACHE:
        _CACHE["nc"] = _build_nc()
    return _CACHE["nc"]


def _make_shards(logits):
    logits = np.asarray(logits).astype(np.float16)
    shards = []
    for k in range(NCORES):
        arr = np.ones((NSEG, P, 20, SEGW), dtype=np.float16)
        segs_per_unit = W // SEGW
        for s in range(NSEG):
            n, sl = UNITS[UPC * k + s // segs_per_unit]
            b = s % segs_per_unit
            blk = logits[n, :, sl * P:(sl + 1) * P, b * SEGW:(b + 1) * SEGW]
            arr[s, :, 0:19, :] = blk.transpose(1, 0, 2)
        shards.append(arr.reshape(NSEG, P, 20, WB, NGRP))
    return shards


def _finish(results):
    per_n = np.zeros((N, C, 2 * C + 1), dtype=np.float64)
    for k in range(NCORES):
        st = np.asarray(results[k]["stats"], dtype=np.float64)  # (128, 39)
        acc = np.zeros((C, 2 * C + 1))
        for g in range(NGRP):
            acc += st[32 * g:32 * g + C, :]
        per_n[k // UPC] += acc
    S1 = per_n[:, :, 0:19].sum(axis=2)
    cnt = per_n[:, :, 19]
    S2 = per_n[:, :, 20:39].sum(axis=2)
    K = np.maximum(cnt, 1.0) * C
    sq_dev = np.maximum(S2 - S1 * S1 / K, 0.0)
    norms = np.where(cnt > 0, np.sqrt(sq_dev), 0.0)
    loss = norms.mean(axis=0).sum()
    return np.array(loss, dtype=np.float32)


def kernel(**inputs):
    _ensure_ntff_hook()
    from concourse.bass_utils import run_bass_kernel_spmd

    logits = np.asarray(inputs["logits"])
    assert logits.shape == (N, C, H, W), logits.shape
    nc = _get_nc()
    shards = _make_shards(logits)
    in_maps = [{"x": shards[k]} for k in range(NCORES)]
    res = run_bass_kernel_spmd(nc, in_maps, list(range(NCORES)))
    return _finish(res.results)
